# revision 1
# baseline (speedup 1.0000x reference)
# Trainium2 Bass kernel for nn_NetSparse1 (topk_masking).
#
# Computes: log_softmax( relu(x @ (w1*m1).T) @ (w2*m2).T ) where m1/m2 are
# top-50%-|score| masks (GetSubnetEP semantics, stable-sort tie handling).
#
# Strategy (data-parallel over 8 NeuronCores, batch dim sharded):
#   host: transpose/cast inputs (xT, w1T bf16, scores1T f32, ...), compute the
#         exact top-k threshold t per layer (k-th order statistic of |scores|)
#         plus stable-sort tie corrections (tie entries the reference drops are
#         zeroed directly in the bf16 weight copies).
#   device (per core, 2048 batch rows):
#     phase A: stream scores1T/w1T, mask = (|s| >= t), w1m = mask * w1 (bf16),
#              kept resident in SBUF.
#     main:    for each 512-batch block: 64x hidden chunks of
#              psum[128h,512b] += w1m_chunk.T @ xT_chunk  (7 K-chunks of 128),
#              relu->bf16, then logitsT[10,512] += w2m_chunk.T @ h_chunk.
#     epilog:  PE-transpose logitsT to [128b,10], log_softmax along free dim,
#              DMA out.
# No collectives needed; host concatenates the 8 per-core outputs.

import numpy as np
import ml_dtypes

import concourse.bass as bass
import concourse.tile as tile
from concourse import bacc, mybir
from concourse.bass_utils import run_bass_kernel_spmd
from concourse.masks import make_identity

N_CORES = 8
B = 16384
BC = B // N_CORES      # 2048 batch rows per core
IN_DIM = 784
HIDDEN = 8192
OUT_DIM = 10
SPARSITY = 0.5

P = 128
KC = 7                 # ceil(784/128) contraction chunks
K_LAST = IN_DIM - 6 * P  # 16
HC = HIDDEN // P       # 64 hidden chunks
BB = 512               # batch block (PSUM free dim)
NBB = BC // BB         # 4
CB = 1024              # phase-A column piece over hidden
NCB = HIDDEN // CB     # 8

F32 = mybir.dt.float32
BF16 = mybir.dt.bfloat16

_BF16 = ml_dtypes.bfloat16


def _build_nc():
    nc = bacc.Bacc("TRN2")

    xT = nc.dram_tensor("xT", (IN_DIM, BC), BF16, kind="ExternalInput")
    w1T = nc.dram_tensor("w1T", (IN_DIM, HIDDEN), BF16, kind="ExternalInput")
    s1T = nc.dram_tensor("s1T", (IN_DIM, HIDDEN), F32, kind="ExternalInput")
    w2T = nc.dram_tensor("w2T", (HIDDEN, OUT_DIM), BF16, kind="ExternalInput")
    s2T = nc.dram_tensor("s2T", (HIDDEN, OUT_DIM), F32, kind="ExternalInput")
    ths = nc.dram_tensor("ths", (1, 2), F32, kind="ExternalInput")
    out = nc.dram_tensor("out", (BC, OUT_DIM), F32, kind="ExternalOutput")

    with tile.TileContext(nc) as tc:
        with (
            tc.tile_pool(name="singles", bufs=1) as singles,
            tc.tile_pool(name="wres", bufs=1) as wres,
            tc.tile_pool(name="stream", bufs=2) as stream,
            tc.tile_pool(name="w2p", bufs=1) as w2p,
            tc.tile_pool(name="hpool", bufs=4) as hpool,
            tc.tile_pool(name="opool", bufs=4) as opool,
            tc.tile_pool(name="psh", bufs=2, space=bass.MemorySpace.PSUM) as psh,
            tc.tile_pool(name="psl", bufs=2, space=bass.MemorySpace.PSUM) as psl,
            tc.tile_pool(name="pst", bufs=2, space=bass.MemorySpace.PSUM) as pst,
        ):
            # thresholds broadcast across partitions: [128, 2]
            t_bc = singles.tile([P, 2], F32, tag="t_bc")
            nc.sync.dma_start(t_bc, bass.AP(ths, 0, [[0, P], [1, 2]]))

            # zero bias for activations
            zb = singles.tile([P, 1], F32, tag="zb")
            nc.vector.memset(zb, 0.0)

            # identity for PE transpose
            ident = singles.tile([P, P], F32, tag="ident")
            make_identity(nc, ident[:])

            # resident xT tiles: 7 x [128, 2048] bf16
            xs = []
            for kc in range(KC):
                pk = P if kc < KC - 1 else K_LAST
                xt = wres.tile([P, BC], BF16, tag=f"x_{kc}")
                if pk < P:
                    nc.vector.memset(xt, 0.0)
                nc.sync.dma_start(xt[:pk, :], xT[kc * P : kc * P + pk, :])
                xs.append(xt)

            # masked w2 (resident): [128, 64, 10] bf16
            w2m = singles.tile([P, HC, OUT_DIM], BF16, tag="w2m")
            s2_t = w2p.tile([P, HC, OUT_DIM], F32, tag="s2_t")
            w2_t = w2p.tile([P, HC, OUT_DIM], BF16, tag="w2_t")
            ge2 = w2p.tile([P, HC, OUT_DIM], BF16, tag="ge2")
            nc.sync.dma_start(s2_t, s2T[:].rearrange("(c p) o -> p c o", p=P))
            nc.sync.dma_start(w2_t, w2T[:].rearrange("(c p) o -> p c o", p=P))
            nc.scalar.activation(out=s2_t, in_=s2_t,
                                 func=mybir.ActivationFunctionType.Abs, bias=zb)
            nc.vector.tensor_scalar(out=ge2, in0=s2_t, scalar1=t_bc[:, 1:2],
                                    scalar2=None, op0=mybir.AluOpType.is_ge)
            nc.vector.tensor_mul(w2m, ge2, w2_t)

            # phase A: masked w1, resident as 7x8 pieces of [128, 1024] bf16
            w1m = [[None] * NCB for _ in range(KC)]
            for cb in range(NCB):
                for kc in range(KC):
                    pk = P if kc < KC - 1 else K_LAST
                    dst = wres.tile([P, CB], BF16, tag=f"w1m_{kc}_{cb}")
                    if pk < P:
                        nc.vector.memset(dst, 0.0)
                    sc = stream.tile([P, CB], F32, tag="sc")
                    nc.sync.dma_start(
                        sc[:pk], s1T[kc * P : kc * P + pk, cb * CB : (cb + 1) * CB])
                    nc.scalar.activation(out=sc[:pk], in_=sc[:pk],
                                         func=mybir.ActivationFunctionType.Abs,
                                         bias=zb[:pk])
                    ge = stream.tile([P, CB], BF16, tag="ge")
                    nc.vector.tensor_scalar(out=ge[:pk], in0=sc[:pk],
                                            scalar1=t_bc[:pk, 0:1], scalar2=None,
                                            op0=mybir.AluOpType.is_ge)
                    wt = stream.tile([P, CB], BF16, tag="wt")
                    nc.sync.dma_start(
                        wt[:pk], w1T[kc * P : kc * P + pk, cb * CB : (cb + 1) * CB])
                    nc.vector.tensor_mul(dst[:pk], ge[:pk], wt[:pk])
                    w1m[kc][cb] = dst

            # main compute
            hc_per_cb = CB // P  # 8
            for bb in range(NBB):
                lg_ps = psl.tile([OUT_DIM, BB], F32, tag="lg_ps")
                for hc in range(HC):
                    w1m_t = w1m[0][hc // hc_per_cb]
                    off = (hc % hc_per_cb) * P
                    ph = psh.tile([P, BB], F32, tag="ph")
                    for kc in range(KC):
                        nc.tensor.matmul(
                            ph,
                            w1m[kc][hc // hc_per_cb][:, off : off + P],
                            xs[kc][:, bb * BB : (bb + 1) * BB],
                            start=(kc == 0),
                            stop=(kc == KC - 1),
                        )
                    ht = hpool.tile([P, BB], BF16, tag="ht")
                    nc.scalar.activation(out=ht, in_=ph,
                                         func=mybir.ActivationFunctionType.Relu,
                                         bias=zb)
                    nc.tensor.matmul(
                        lg_ps,
                        w2m[:, hc, :],
                        ht,
                        start=(hc == 0),
                        stop=(hc == HC - 1),
                    )

                # log_softmax epilogue for this 512-batch block
                lg_sb = opool.tile([OUT_DIM, BB], F32, tag="lg_sb")
                nc.vector.tensor_copy(lg_sb, lg_ps)
                for bs in range(BB // P):
                    pt = pst.tile([P, OUT_DIM], F32, tag="pt")
                    nc.tensor.transpose(pt, lg_sb[:, bs * P : (bs + 1) * P],
                                        ident[:OUT_DIM, :OUT_DIM])
                    mx = opool.tile([P, 1], F32, tag="mx")
                    nc.vector.reduce_max(out=mx, in_=pt, axis=mybir.AxisListType.X)
                    xm = opool.tile([P, OUT_DIM], F32, tag="xm")
                    nc.vector.tensor_scalar(out=xm, in0=pt, scalar1=mx,
                                            scalar2=None,
                                            op0=mybir.AluOpType.subtract)
                    e = opool.tile([P, OUT_DIM], F32, tag="e")
                    s = opool.tile([P, 1], F32, tag="s")
                    nc.scalar.activation(out=e, in_=xm,
                                         func=mybir.ActivationFunctionType.Exp,
                                         bias=zb, accum_out=s)
                    ls = opool.tile([P, 1], F32, tag="ls")
                    nc.scalar.activation(out=ls, in_=s,
                                         func=mybir.ActivationFunctionType.Ln,
                                         bias=zb)
                    ot = opool.tile([P, OUT_DIM], F32, tag="ot")
                    nc.vector.tensor_scalar(out=ot, in0=xm, scalar1=ls,
                                            scalar2=None,
                                            op0=mybir.AluOpType.subtract)
                    row0 = bb * BB + bs * P
                    nc.sync.dma_start(out[row0 : row0 + P, :], ot)

    nc.compile()
    return nc


_NC = None


def _get_nc():
    global _NC
    if _NC is None:
        _NC = _build_nc()
    return _NC


def _topk_threshold_and_fix(scores, wT_bf16):
    """Exact GetSubnetEP mask via threshold + stable-sort tie correction.

    Returns t such that the device mask (|s| >= t) keeps a superset of the
    reference's kept set; tie entries the reference drops (ties at t with the
    smallest flat indices) are zeroed directly in wT_bf16 (transposed layout),
    making the effective masked weights exact.
    """
    a = np.abs(np.asarray(scores, dtype=np.float32)).ravel()
    n = a.size
    j = int((1.0 - SPARSITY) * n)
    t = np.partition(a, j)[j]
    lt = int((a < t).sum())
    ties = np.flatnonzero(a == t)
    dropped = ties[: j - lt]
    ncols = scores.shape[1]
    r = dropped // ncols  # original row (output-channel) index
    c = dropped % ncols   # original col index
    wT_bf16[c, r] = 0
    # sanity: device-kept count must equal reference-kept count
    assert int((a >= t).sum()) - len(dropped) == n - j
    return np.float32(t)


def _prepare_inputs(x, w1, scores1, w2, scores2):
    x = np.asarray(x, dtype=np.float32)
    w1 = np.asarray(w1, dtype=np.float32)
    scores1 = np.asarray(scores1, dtype=np.float32)
    w2 = np.asarray(w2, dtype=np.float32)
    scores2 = np.asarray(scores2, dtype=np.float32)

    w1T = np.ascontiguousarray(w1.T).astype(_BF16)   # [784, 8192]
    w2T = np.ascontiguousarray(w2.T).astype(_BF16)   # [8192, 10]
    t1 = _topk_threshold_and_fix(scores1, w1T)
    t2 = _topk_threshold_and_fix(scores2, w2T)

    s1T = np.ascontiguousarray(scores1.T)            # [784, 8192] f32
    s2T = np.ascontiguousarray(scores2.T)            # [8192, 10] f32
    xTb = np.ascontiguousarray(x.T).astype(_BF16)    # [784, 16384]
    ths = np.array([[t1, t2]], dtype=np.float32)

    common = {"w1T": w1T, "s1T": s1T, "w2T": w2T, "s2T": s2T, "ths": ths}
    in_maps = []
    for c in range(N_CORES):
        m = dict(common)
        m["xT"] = np.ascontiguousarray(xTb[:, c * BC : (c + 1) * BC])
        in_maps.append(m)
    return in_maps


def run(inputs, trace=False, **kwargs):
    """Run the kernel; returns (output ndarray, BassKernelResults)."""
    nc = _get_nc()
    in_maps = _prepare_inputs(**inputs)
    res = run_bass_kernel_spmd(nc, in_maps, core_ids=list(range(N_CORES)),
                               trace=trace, **kwargs)
    outp = np.concatenate([r["out"] for r in res.results], axis=0)
    return np.ascontiguousarray(outp.astype(np.float32)), res


def kernel(x, w1, scores1, w2, scores2):
    outp, _ = run(dict(x=x, w1=w1, scores1=scores1, w2=w2, scores2=scores2))
    return outp
